# revision 24
# baseline (speedup 1.0000x reference)
"""Trainium2 Bass kernel for nn_MultiHeadAttention_53266184405720.

Key structural fact: the reference does a raw ``.reshape(h, -1, d)`` on the
[4096, 512] projection output, so "head" h consumes exactly projection rows
[512h, 512h+512) — i.e. sequence rows [512h, 512h+512).  The whole module is
block-diagonal over 512-row sequence blocks: core h computes output rows
[512h, 512h+512) from input rows [512h, 512h+512) plus the (replicated)
weights.  No cross-core communication is needed.

Within a block, with the permutation r~ = c*512 + s (c = column-block of the
projection, s = row), head-reshaped Q/K/V become column-block stacks of the
projection, softmax is permutation-invariant over keys, and the context
unpermutes back into the output projection's contraction.  The transposed
projection layout [64, 512] per column-block c therefore yields every
attention operand as a zero-cost sub-AP.

Perf choices (HW-measured):
 - fp32 matmul = 4 cyc/row; bf16 = 1 cyc/row with fast weight loads -> bf16
   for scores / attention*V / output projection, f32r for input projections.
 - K=64 score matmuls pack 2-per-PE via tile_position rows (0,0)/(64,0).
 - THE EXP SPLIT: exp is the second wall (ACT = 1 elem/lane/cyc @1.2GHz =
   ~109us/core floor for 16.8M scores).  A custom DVE op (EXP2_BITS_ANT)
   computes exp as int16 bf16-bit-pattern: scores arrive pre-scaled by
   16*log2(e) (folded into the Q/K casts), the op extracts the binade
   fraction with the fp32 big-constant RNE trick (+1.5*2^30), applies a
   minimax quadratic mantissa correction (~0.9% rms), and the i16 result is
   bit-cast to bf16.  Score groups alternate ACT (exact exp) / DVE (custom
   op) so both engines chew the exp wall concurrently.
 - softmax denominator rides as a ones-column in the V operand (row 64 of
   the ctx accumulator); 1/denom via reciprocal_approx_fast (~5x faster);
   broadcast across partitions via a DRAM-bounce DMA; last chunk broadcasts
   via a K=1 ones-matmul instead (PE idle then, DMA latency not).
"""

import numpy as np

SEQ = 4096
D = 64
HEADS = 8
B = SEQ // HEADS  # 512 rows per core
N_CORES = 8

_BUILT = None
_EXP_OP = None

# exp-op constants: scores arrive pre-scaled so that score_c = 16*log2e * s_raw
# (exp argument is s_raw/8 -> w_code = 128 * log2(exp-arg... ) = score_c).
SCALE_C = 16.0 * np.log2(np.e)          # 23.083120654223414
SCP = float(np.sqrt(SCALE_C))           # folded into both Q and K casts
C1BIG = 1.5 * 2**30
# minimax quadratic fit of the round-based (kinked) mantissa correction
QC = -0.00247243   # quadratic coef (s0 / C0)
LC = 0.013384      # linear coef (imm2 / C2)
CC = -2.8858       # constant coef -> folded into the exponent anchor (C3)
ANCHOR = 16256.0 + CC

DVE_PATTERN_NUM = 6    # DVE gets 6 of every 16 exp groups
DVE_PATTERN_DEN = 16


def _get_exp_op():
    global _EXP_OP
    if _EXP_OP is not None:
        return _EXP_OP
    import dataclasses
    from concourse.dve_spec import Spec, Src0, C0, C1, C2, C3, lower
    from concourse.dve_ops import (
        DveOp, OPS, CUSTOM_DVE_SPECS, _SUB_OPCODE_FOR_NAME,
        _CUSTOM_DVE_ROW_BASE, DveOpSpec, _spill_c3_to_src1,
    )
    name = "EXP2_BITS_ANT"
    if name in _SUB_OPCODE_FOR_NAME:
        _EXP_OP = next(o for o in OPS if o.name == name)
        return _EXP_OP
    _w = Src0 + C1
    _v = _w - C1              # RNE to multiples of 128 (fp32 big-const trick)
    _g = Src0 - _v            # signed binade frac in code units, [-64, 64)
    _p = _g * (_g * C0 + C2)  # quadratic mantissa correction
    _t = (Src0 + _p) + C3     # C3 (latched from in1) = exponent anchor

    def _ref(in0, in1, s0, s1, imm2):
        w = (in0 + np.float32(s1)).astype(np.float32)
        v = (w - np.float32(s1)).astype(np.float32)
        g = in0 - v
        c3 = in1[:, :1] if in1 is not None else 0.0
        return (in0 + g * (g * np.float32(s0) + np.float32(imm2))) + c3

    op = DveOp(name, Spec(body=_spill_c3_to_src1(_t), reference=_ref),
               subdim=False, uops_sha={})
    row = _CUSTOM_DVE_ROW_BASE + len(OPS)
    shas = {}
    for ver in ("v3", "v4"):
        spec_l = DveOpSpec(name=name, opcode=row,
                           uops=lower(op.spec, ver=ver), rd1_en=True)
        shas[ver] = spec_l.sha(ver)
    op = dataclasses.replace(op, uops_sha=shas)
    OPS.append(op)
    CUSTOM_DVE_SPECS[name] = op.spec
    _SUB_OPCODE_FOR_NAME[name] = row
    _EXP_OP = op
    return op


def _build():
    import concourse.bass as bass
    import concourse.tile as tile
    from concourse import bacc, mybir
    from concourse.masks import make_identity

    exp_op = _get_exp_op()

    f32 = mybir.dt.float32
    f32r = mybir.dt.float32r
    bf16 = mybir.dt.bfloat16
    i16 = mybir.dt.int16
    AF = mybir.ActivationFunctionType

    nc = bacc.Bacc(
        "TRN2",
        target_bir_lowering=False,
        debug=False,
        enable_asserts=True,
        num_devices=N_CORES,
    )

    q = nc.dram_tensor("q", [B, D], f32, kind="ExternalInput").ap()
    k = nc.dram_tensor("k", [B, D], f32, kind="ExternalInput").ap()
    v = nc.dram_tensor("v", [B, D], f32, kind="ExternalInput").ap()
    qw_w = nc.dram_tensor("qw_w", [D, 512], f32, kind="ExternalInput").ap()
    qw_b = nc.dram_tensor("qw_b", [512], f32, kind="ExternalInput").ap()
    kw_w = nc.dram_tensor("kw_w", [D, 512], f32, kind="ExternalInput").ap()
    kw_b = nc.dram_tensor("kw_b", [512], f32, kind="ExternalInput").ap()
    vw_w = nc.dram_tensor("vw_w", [D, 512], f32, kind="ExternalInput").ap()
    vw_b = nc.dram_tensor("vw_b", [512], f32, kind="ExternalInput").ap()
    ow_w = nc.dram_tensor("ow_w", [512, D], f32, kind="ExternalInput").ap()
    ow_b = nc.dram_tensor("ow_b", [D], f32, kind="ExternalInput").ap()
    out = nc.dram_tensor("out", [B, D], f32, kind="ExternalOutput").ap()

    ACT_SCALE = float(0.125 / SCALE_C)  # exp(scale * score_c) = exp(s/8)

    def matmul_noldw(out, lhsT, rhs, start, stop, tile_position):
        """InstMatmult with ldweights=False: weights must already be resident
        (loaded by a preceding nc.tensor.ldweights covering this tile)."""
        te = nc.tensor
        keep_dims = frozenset({0})
        ifmap_ap = te.lower_ap(rhs.opt(keep_dims), opt=False)
        weights_ap = te.lower_ap(lhsT.opt(keep_dims), opt=False, for_matmul_weights=True)
        out_ap = te.lower_ap(out)
        return te.add_instruction(
            mybir.InstMatmult(
                name=nc.get_next_instruction_name(),
                replication_resolution=0,
                replication_shift_amnt=0,
                replication_num_rows=0,
                start_tensor_calc=start,
                stop_tensor_calc=stop,
                ins=[ifmap_ap, weights_ap],
                outs=[out_ap],
                tile_position=tile_position,
                tile_size=(64, 128),
                ldweights=False,
            )
        )

    with tile.TileContext(nc) as tc:
        with (
            tc.tile_pool(name="persist", bufs=1) as persist,
            tc.tile_pool(name="inp", bufs=3) as inp,
            tc.tile_pool(name="epool", bufs=8) as epool,
            tc.tile_pool(name="norm", bufs=3) as normp,
            tc.tile_pool(name="outp", bufs=2) as outp,
            tc.tile_pool(name="ps_sc", bufs=3, space="PSUM") as ps_sc,
            tc.tile_pool(name="ps_ctx", bufs=2, space="PSUM") as ps_ctx,
            tc.tile_pool(name="dramp", bufs=2, space="DRAM") as dramp,
        ):
            # ---- interleaved input/weight DMAs ----
            qT = persist.tile([65, 512], bf16, tag="qT")
            kT = persist.tile([65, 512], bf16, tag="kT")
            vT = persist.tile([65, 512], bf16, tag="vT")
            xins = {}
            wstgs = {}
            for name, x_d, w_d, b_d, eng in (
                ("q", q, qw_w, qw_b, nc.sync),
                ("k", k, kw_w, kw_b, nc.scalar),
                ("v", v, vw_w, vw_b, nc.gpsimd),
            ):
                xin = inp.tile([128, 4, 64], f32, tag="xin", name=f"xin_{name}")
                xr = x_d.rearrange("(t p) d -> p t d", p=128)
                for t in range(4):
                    eng.dma_start(out=xin[:, t, :], in_=xr[:, t, :])
                xins[name] = xin
                stg = inp.tile([65, 512], f32, tag="wstg", name=f"wstg_{name}")
                eng.dma_start(out=stg[0:64, :], in_=w_d)
                eng.dma_start(out=stg[64:65, :], in_=b_d[None, :])
                wstgs[name] = stg

            # prefetch output-projection weights (needed only at the tail,
            # but the DMA queues are idle here)
            ow_stg = persist.tile([64, 8, 64], f32, tag="ow_stg")
            nc.scalar.dma_start(
                out=ow_stg, in_=ow_w.rearrange("(c d) j -> d c j", d=64)
            )
            owb_stg = persist.tile([1, 64], f32, tag="owb_stg")
            nc.scalar.dma_start(out=owb_stg, in_=ow_b[None, :])

            # ---- constants (gpsimd/ACT, overlap the DMAs) ----
            ident = persist.tile([128, 128], f32, tag="ident")
            make_identity(nc, ident)
            ones_a = persist.tile([65, 64], f32, tag="ones_a")
            nc.gpsimd.memset(ones_a, 1.0)
            ones_b = persist.tile([1, 128], bf16, tag="ones_b")
            nc.gpsimd.memset(ones_b, 1.0)
            ones_row = persist.tile([1, 512], f32, tag="ones_row")
            nc.gpsimd.memset(ones_row, 1.0)
            # C3 latch for the custom exp op (exponent anchor per partition)
            anchor = persist.tile([128, 1], f32, tag="anchor")
            nc.gpsimd.memset(anchor, float(ANCHOR))
            # dummy exp to pull the ACT table load into the setup phase
            warm = persist.tile([1, 16], f32, tag="warm")
            nc.scalar.activation(warm, ones_row[:, 0:16], AF.Exp, scale=1.0)

            qw_aug = persist.tile([65, 512], bf16, tag="qw_aug")
            kw_aug = persist.tile([65, 512], bf16, tag="kw_aug")
            vw_aug = persist.tile([65, 512], bf16, tag="vw_aug")
            # weight casts only need the wstg DMAs -> run before the
            # transpose chains so they're off the projection critical path
            nc.vector.tensor_copy(out=qw_aug, in_=wstgs["q"])
            nc.vector.tensor_copy(out=kw_aug, in_=wstgs["k"])
            nc.vector.tensor_copy(out=vw_aug, in_=wstgs["v"])

            def transposes(name, xT):
                nc.vector.tensor_copy(out=xT[64:65, :], in_=ones_row)
                for t in range(4):
                    tp = ps_sc.tile([64, 128], f32, tag="sc", name=f"tp_{name}{t}")
                    nc.tensor.transpose(tp, xins[name][:, t, :], ident)
                    nc.vector.tensor_copy(
                        out=xT[0:64, 128 * t : 128 * t + 128], in_=tp
                    )

            # ---- Q chain: transposes, projections, chunk-0 dup ----
            # Qdup/KpT casts scale by SCP each so scores come out of the PE
            # pre-scaled by SCALE_C (code units for the DVE exp op).
            transposes("q", qT)

            Qdup = persist.tile([128, 4096], bf16, tag="Qdup")

            def qproj(m, eng):
                pool, tg = (ps_sc, "sc") if m % 2 == 0 else (ps_ctx, "ctx")
                ps = pool.tile([128, 512], f32, tag=tg, name=f"qp{m}")
                nc.tensor.matmul(
                    ps,
                    lhsT=qw_aug[:, 128 * m : 128 * m + 128],
                    rhs=qT[:],
                    start=True,
                    stop=True,
                )
                ce, co = 2 * m, 2 * m + 1
                dst_e = Qdup[0:64, 512 * ce : 512 * ce + 512]
                dst_o = Qdup[64:128, 512 * co : 512 * co + 512]
                if eng == "act":
                    nc.scalar.activation(dst_e, ps[0:64, :], AF.Copy, scale=SCP)
                    nc.vector.tensor_scalar_mul(dst_o, ps[64:128, :], SCP)
                else:
                    nc.vector.tensor_scalar_mul(dst_e, ps[0:64, :], SCP)
                    nc.scalar.activation(dst_o, ps[64:128, :], AF.Copy, scale=SCP)

            qproj(0, "dve")
            nc.sync.dma_start(out=Qdup[64:128, 0:512], in_=Qdup[0:64, 0:512])
            nc.sync.dma_start(out=Qdup[0:64, 512:1024], in_=Qdup[64:128, 512:1024])
            for m in range(1, 4):
                qproj(m, "act" if m % 2 == 0 else "dve")
                ce, co = 2 * m, 2 * m + 1
                nc.sync.dma_start(
                    out=Qdup[64:128, 512 * ce : 512 * ce + 512],
                    in_=Qdup[0:64, 512 * ce : 512 * ce + 512],
                )
                nc.sync.dma_start(
                    out=Qdup[0:64, 512 * co : 512 * co + 512],
                    in_=Qdup[64:128, 512 * co : 512 * co + 512],
                )

            # ---- K chain ----
            transposes("k", kT)
            KpT = []
            for g in range(4):
                pool, tg = (ps_sc, "sc") if g % 2 == 0 else (ps_ctx, "ctx")
                ps = pool.tile([128, 512], f32, tag=tg, name=f"kp{g}")
                nc.tensor.matmul(
                    ps,
                    lhsT=kw_aug[:, 128 * g : 128 * g + 128],
                    rhs=kT[:],
                    start=True,
                    stop=True,
                )
                sb = persist.tile([128, 512], bf16, tag=f"KpT{g}")
                if g % 2 == 0:
                    nc.scalar.activation(sb, ps, AF.Copy, scale=SCP)
                else:
                    nc.vector.tensor_scalar_mul(sb, ps, SCP)
                KpT.append(sb)

            # ---- V chain ----
            transposes("v", vT)
            Va = []
            for u in range(4):
                pool, tg = (ps_sc, "sc") if u % 2 == 0 else (ps_ctx, "ctx")
                ps = pool.tile([128, 512], f32, tag=tg, name=f"vp{u}")
                nc.tensor.matmul(
                    ps,
                    lhsT=vT[:, 128 * u : 128 * u + 128],
                    rhs=vw_aug[:],
                    start=True,
                    stop=True,
                )
                va = persist.tile([128, 520], bf16, tag=f"Va{u}")
                nc.gpsimd.memset(va, 1.0)
                vdst = va[:].rearrange("p (c jj) -> p c jj", c=8)[:, :, 0:64]
                vsrc = ps[:].rearrange("p (c j) -> p c j", c=8)
                if u % 2 == 0:
                    nc.scalar.copy(out=vdst, in_=vsrc)
                else:
                    nc.vector.tensor_copy(out=vdst, in_=vsrc)
                Va.append(va)

            # ---- main attention loop ----
            # units issued as packed pairs (kt=8g+u rows 0-63, kt=8g+4+u rows
            # 64-127); exp groups of gsize=2 units = [128, 1024] PSUM (2 banks,
            # 3 bufs so PE / ACT / DVE each own one in flight).
            unit_order = []
            for g in range(4):
                for u in range(4):
                    unit_order.append(8 * g + u)
                    unit_order.append(8 * g + 4 + u)

            ctxN = persist.tile([64, 4096], bf16, tag="ctxN")
            ctx_tiles = {}
            av_issued = {r1c: 0 for r1c in range(8)}
            pending = []  # (r1c, e_tile, units[(slot, kt)])
            AV_DELAY = 4
            gsize = 2

            dve_acc = [0]

            def pick_engine():
                dve_acc[0] += DVE_PATTERN_NUM
                if dve_acc[0] >= DVE_PATTERN_DEN:
                    dve_acc[0] -= DVE_PATTERN_DEN
                    return "dve"
                return "act"

            def emit_avs(rec_):
                r1c, e_bf, units = rec_
                ctx_ps = ctx_tiles[r1c]
                for slot, kt in units:
                    c, u = kt // 4, kt % 4
                    i = av_issued[r1c]
                    nc.tensor.matmul(
                        ctx_ps,
                        lhsT=Va[u][:, 65 * c : 65 * c + 65],
                        rhs=e_bf[:, 512 * slot : 512 * slot + 512],
                        start=(i == 0),
                        stop=(i == 31),
                    )
                    av_issued[r1c] = i + 1

            def normalize(r1c):
                # custom-DVE ops (reciprocal_approx_fast) are broken at
                # base_partition != 0, so broadcast the RAW denominator to a
                # base-0 [64, 512] first, then take the fast reciprocal there.
                ctx_ps = ctx_tiles.pop(r1c)
                den_sb = normp.tile([65, 512], f32, tag="densb")
                nc.vector.tensor_copy(out=den_sb[64:65, :], in_=ctx_ps[64:65, :])
                if r1c == 7:
                    # tail chunk: PE idle -> broadcast via K=1 ones matmul
                    den_bc_ps = ps_sc.tile([64, 512], f32, tag="sc", name="repl")
                    nc.tensor.matmul(
                        den_bc_ps,
                        lhsT=ones_a[64:65, :],
                        rhs=den_sb[64:65, :],
                        start=True,
                        stop=True,
                        tile_position=(64, 0),
                    )
                    den_src = den_bc_ps
                else:
                    rec_d = dramp.tile([1, 512], f32, tag="rec_d")
                    nc.sync.dma_start(out=rec_d, in_=den_sb[64:65, :])
                    den_bc = normp.tile([64, 512], f32, tag="denbc")
                    rd = rec_d[0, :]
                    nc.sync.dma_start(
                        out=den_bc,
                        in_=bass.AP(
                            tensor=rd.tensor,
                            offset=rd.offset,
                            ap=[[0, 64]] + list(rd.ap),
                        ),
                    )
                    den_src = den_bc
                rec_bc = normp.tile([64, 512], f32, tag="recbc")
                nc.vector.reciprocal_approx_fast(rec_bc, den_src)
                nc.vector.tensor_mul(
                    out=ctxN[:, 512 * r1c : 512 * r1c + 512],
                    in0=ctx_ps[0:64, :],
                    in1=rec_bc,
                )

            for r1c in range(8):
                ctx_tiles[r1c] = ps_ctx.tile(
                    [65, 512], f32, tag="ctx", name=f"ctx{r1c}"
                )
                group_tile = None
                group_units = []

                def flush(r1c=r1c):
                    nonlocal group_tile, group_units
                    if not group_units:
                        return
                    n = len(group_units)
                    e = epool.tile([128, 1024], bf16, tag="e")
                    if pick_engine() == "act":
                        nc.scalar.activation(
                            e[:, : 512 * n],
                            group_tile[:, : 512 * n],
                            AF.Exp,
                            scale=ACT_SCALE,
                        )
                    else:
                        nc.vector._custom_dve(
                            exp_op,
                            out=e[:, : 512 * n].bitcast(i16),
                            in0=group_tile[:, : 512 * n],
                            in1=anchor,
                            s0=QC,
                            s1=float(C1BIG),
                            imm2=LC,
                        )
                    pending.append((r1c, e, group_units))
                    group_tile = None
                    group_units = []
                    # drain four groups every fourth flush: fewer score<->AV
                    # transitions on the PE (each costs a ~100ns LDWEIGHTS
                    # bubble from row-group conflicts)
                    if len(pending) > AV_DELAY + 1:
                        for _ in range(2):
                            rec_ = pending.pop(0)
                            emit_avs(rec_)
                            if av_issued[rec_[0]] == 32:
                                normalize(rec_[0])

                for pi in range(16):
                    kt_a = unit_order[2 * pi]
                    kt_b = unit_order[2 * pi + 1]
                    for kt, half in ((kt_a, 0), (kt_b, 1)):
                        if group_tile is None:
                            group_tile = ps_sc.tile([128, 1024], f32, tag="sc")
                        slot = len(group_units)
                        c, u = kt // 4, kt % 4
                        g = c // 2
                        rowpos = 64 * (c % 2)
                        nc.tensor.matmul(
                            group_tile[:, 512 * slot : 512 * slot + 512],
                            lhsT=KpT[g][
                                rowpos : rowpos + 64, 128 * u : 128 * u + 128
                            ],
                            rhs=Qdup[rowpos : rowpos + 64, 512 * r1c : 512 * r1c + 512],
                            start=True,
                            stop=True,
                            tile_position=(rowpos, 0),
                        )
                        group_units.append((slot, kt))
                        if len(group_units) == gsize:
                            flush()
                flush()
            while pending:
                rec_ = pending.pop(0)
                emit_avs(rec_)
                if av_issued[rec_[0]] == 32:
                    normalize(rec_[0])

            # ---- output projection (bf16) ----
            ow_sb = persist.tile([64, 512], bf16, tag="ow_sb")
            nc.vector.tensor_copy(
                out=ow_sb, in_=ow_stg.rearrange("d c j -> d (c j)")
            )
            owb_sb = persist.tile([1, 64], bf16, tag="owb_sb")
            nc.vector.tensor_copy(out=owb_sb, in_=owb_stg)
            ob = outp.tile([128, 4, 64], f32, tag="ob")
            # t0..t2 accumulate chunks 0..6 in separate PSUM tiles while the
            # chunk-7 normalize chain runs on the DVE; only the final c=7
            # matmuls (and t3) wait on it.
            ops = {}
            for t in range(3):
                ops[t] = ps_sc.tile([128, 64], f32, tag="sc", name=f"op{t}")
                for c in range(7):
                    nc.tensor.matmul(
                        ops[t],
                        lhsT=ctxN[:, 512 * c + 128 * t : 512 * c + 128 * t + 128],
                        rhs=ow_sb[:, 64 * c : 64 * c + 64],
                        start=(c == 0),
                        stop=False,
                    )
            for t in range(3):
                nc.tensor.matmul(
                    ops[t],
                    lhsT=ctxN[:, 512 * 7 + 128 * t : 512 * 7 + 128 * t + 128],
                    rhs=ow_sb[:, 64 * 7 : 64 * 7 + 64],
                    start=False,
                    stop=False,
                )
                nc.tensor.matmul(
                    ops[t], lhsT=ones_b, rhs=owb_sb, start=False, stop=True
                )
                nc.vector.tensor_copy(out=ob[:, t, :], in_=ops[t])
            op3 = ps_sc.tile([128, 64], f32, tag="sc", name="op3")
            for c in range(8):
                nc.tensor.matmul(
                    op3,
                    lhsT=ctxN[:, 512 * c + 128 * 3 : 512 * c + 128 * 3 + 128],
                    rhs=ow_sb[:, 64 * c : 64 * c + 64],
                    start=(c == 0),
                    stop=False,
                )
            nc.tensor.matmul(
                op3, lhsT=ones_b, rhs=owb_sb, start=False, stop=True
            )
            nc.vector.tensor_copy(out=ob[:, 3, :], in_=op3)
            nc.sync.dma_start(
                out=out.rearrange("(t p) d -> p t d", p=128), in_=ob
            )

    nc.compile()
    return nc


def _get_built():
    global _BUILT
    if _BUILT is None:
        _BUILT = _build()
    return _BUILT


def _make_in_maps(inputs):
    f32 = np.float32
    full = {k: np.ascontiguousarray(np.asarray(v, dtype=f32)) for k, v in inputs.items()}
    in_maps = []
    for i in range(N_CORES):
        sl = slice(B * i, B * (i + 1))
        in_maps.append(
            {
                "q": full["q"][sl],
                "k": full["k"][sl],
                "v": full["v"][sl],
                "qw_w": full["qw_w"],
                "qw_b": full["qw_b"],
                "kw_w": full["kw_w"],
                "kw_b": full["kw_b"],
                "vw_w": full["vw_w"],
                "vw_b": full["vw_b"],
                "ow_w": full["ow_w"],
                "ow_b": full["ow_b"],
            }
        )
    return in_maps


def kernel(**inputs):
    from concourse.bass_utils import run_bass_kernel_spmd

    nc = _get_built()
    res = run_bass_kernel_spmd(nc, _make_in_maps(inputs), list(range(N_CORES)))
    return np.concatenate([res.results[i]["out"] for i in range(N_CORES)], axis=0)


# revision 26
# speedup vs baseline: 1.0043x; 1.0043x over previous
"""Trainium2 Bass kernel for nn_MultiHeadAttention_53266184405720.

Key structural fact: the reference does a raw ``.reshape(h, -1, d)`` on the
[4096, 512] projection output, so "head" h consumes exactly projection rows
[512h, 512h+512) — i.e. sequence rows [512h, 512h+512).  The whole module is
block-diagonal over 512-row sequence blocks: core h computes output rows
[512h, 512h+512) from input rows [512h, 512h+512) plus the (replicated)
weights.  No cross-core communication is needed.

Within a block, with the permutation r~ = c*512 + s (c = column-block of the
projection, s = row), head-reshaped Q/K/V become column-block stacks of the
projection, softmax is permutation-invariant over keys, and the context
unpermutes back into the output projection's contraction.  The transposed
projection layout [64, 512] per column-block c therefore yields every
attention operand as a zero-cost sub-AP.

Perf choices (HW-measured):
 - fp32 matmul = 4 cyc/row; bf16 = 1 cyc/row with fast weight loads -> bf16
   for scores / attention*V / output projection, f32r for input projections.
 - K=64 score matmuls pack 2-per-PE via tile_position rows (0,0)/(64,0).
 - THE EXP SPLIT: exp is the second wall (ACT = 1 elem/lane/cyc @1.2GHz =
   ~109us/core floor for 16.8M scores).  A custom DVE op (EXP2_BITS_ANT)
   computes exp as int16 bf16-bit-pattern: scores arrive pre-scaled by
   16*log2(e) (folded into the Q/K casts), the op extracts the binade
   fraction with the fp32 big-constant RNE trick (+1.5*2^30), applies a
   minimax quadratic mantissa correction (~0.9% rms), and the i16 result is
   bit-cast to bf16.  Score groups alternate ACT (exact exp) / DVE (custom
   op) so both engines chew the exp wall concurrently.
 - softmax denominator rides as a ones-column in the V operand (row 64 of
   the ctx accumulator); 1/denom via reciprocal_approx_fast (~5x faster);
   broadcast across partitions via a DRAM-bounce DMA; last chunk broadcasts
   via a K=1 ones-matmul instead (PE idle then, DMA latency not).
"""

import numpy as np

SEQ = 4096
D = 64
HEADS = 8
B = SEQ // HEADS  # 512 rows per core
N_CORES = 8

_BUILT = None
_EXP_OP = None

# exp-op constants: scores arrive pre-scaled so that score_c = 16*log2e * s_raw
# (exp argument is s_raw/8 -> w_code = 128 * log2(exp-arg... ) = score_c).
SCALE_C = 16.0 * np.log2(np.e)          # 23.083120654223414
SCP = float(np.sqrt(SCALE_C))           # folded into both Q and K casts
C1BIG = 1.5 * 2**30
# minimax quadratic fit of the round-based (kinked) mantissa correction
QC = -0.00247243   # quadratic coef (s0 / C0)
LC = 0.013384      # linear coef (imm2 / C2)
CC = -2.8858       # constant coef -> folded into the exponent anchor (C3)
ANCHOR = 16256.0 + CC

DVE_PATTERN_NUM = 6    # DVE gets 6 of every 16 exp groups
DVE_PATTERN_DEN = 16


def _get_exp_op():
    global _EXP_OP
    if _EXP_OP is not None:
        return _EXP_OP
    import dataclasses
    from concourse.dve_spec import Spec, Src0, C0, C1, C2, C3, lower
    from concourse.dve_ops import (
        DveOp, OPS, CUSTOM_DVE_SPECS, _SUB_OPCODE_FOR_NAME,
        _CUSTOM_DVE_ROW_BASE, DveOpSpec, _spill_c3_to_src1,
    )
    name = "EXP2_BITS_ANT"
    if name in _SUB_OPCODE_FOR_NAME:
        _EXP_OP = next(o for o in OPS if o.name == name)
        return _EXP_OP
    _w = Src0 + C1
    _v = _w - C1              # RNE to multiples of 128 (fp32 big-const trick)
    _g = Src0 - _v            # signed binade frac in code units, [-64, 64)
    _p = _g * (_g * C0 + C2)  # quadratic mantissa correction
    _t = (Src0 + _p) + C3     # C3 (latched from in1) = exponent anchor

    def _ref(in0, in1, s0, s1, imm2):
        w = (in0 + np.float32(s1)).astype(np.float32)
        v = (w - np.float32(s1)).astype(np.float32)
        g = in0 - v
        c3 = in1[:, :1] if in1 is not None else 0.0
        return (in0 + g * (g * np.float32(s0) + np.float32(imm2))) + c3

    op = DveOp(name, Spec(body=_spill_c3_to_src1(_t), reference=_ref),
               subdim=False, uops_sha={})
    row = _CUSTOM_DVE_ROW_BASE + len(OPS)
    shas = {}
    for ver in ("v3", "v4"):
        spec_l = DveOpSpec(name=name, opcode=row,
                           uops=lower(op.spec, ver=ver), rd1_en=True)
        shas[ver] = spec_l.sha(ver)
    op = dataclasses.replace(op, uops_sha=shas)
    OPS.append(op)
    CUSTOM_DVE_SPECS[name] = op.spec
    _SUB_OPCODE_FOR_NAME[name] = row
    _EXP_OP = op
    return op


def _build():
    import concourse.bass as bass
    import concourse.tile as tile
    from concourse import bacc, mybir
    from concourse.masks import make_identity

    exp_op = _get_exp_op()

    f32 = mybir.dt.float32
    f32r = mybir.dt.float32r
    bf16 = mybir.dt.bfloat16
    i16 = mybir.dt.int16
    AF = mybir.ActivationFunctionType

    nc = bacc.Bacc(
        "TRN2",
        target_bir_lowering=False,
        debug=False,
        enable_asserts=True,
        num_devices=N_CORES,
    )

    q = nc.dram_tensor("q", [B, D], f32, kind="ExternalInput").ap()
    k = nc.dram_tensor("k", [B, D], f32, kind="ExternalInput").ap()
    v = nc.dram_tensor("v", [B, D], f32, kind="ExternalInput").ap()
    qw_w = nc.dram_tensor("qw_w", [D, 512], f32, kind="ExternalInput").ap()
    qw_b = nc.dram_tensor("qw_b", [512], f32, kind="ExternalInput").ap()
    kw_w = nc.dram_tensor("kw_w", [D, 512], f32, kind="ExternalInput").ap()
    kw_b = nc.dram_tensor("kw_b", [512], f32, kind="ExternalInput").ap()
    vw_w = nc.dram_tensor("vw_w", [D, 512], f32, kind="ExternalInput").ap()
    vw_b = nc.dram_tensor("vw_b", [512], f32, kind="ExternalInput").ap()
    ow_w = nc.dram_tensor("ow_w", [512, D], f32, kind="ExternalInput").ap()
    ow_b = nc.dram_tensor("ow_b", [D], f32, kind="ExternalInput").ap()
    out = nc.dram_tensor("out", [B, D], f32, kind="ExternalOutput").ap()

    ACT_SCALE = float(0.125 / SCALE_C)  # exp(scale * score_c) = exp(s/8)

    def matmul_noldw(out, lhsT, rhs, start, stop, tile_position):
        """InstMatmult with ldweights=False: weights must already be resident
        (loaded by a preceding nc.tensor.ldweights covering this tile)."""
        te = nc.tensor
        keep_dims = frozenset({0})
        ifmap_ap = te.lower_ap(rhs.opt(keep_dims), opt=False)
        weights_ap = te.lower_ap(lhsT.opt(keep_dims), opt=False, for_matmul_weights=True)
        out_ap = te.lower_ap(out)
        return te.add_instruction(
            mybir.InstMatmult(
                name=nc.get_next_instruction_name(),
                replication_resolution=0,
                replication_shift_amnt=0,
                replication_num_rows=0,
                start_tensor_calc=start,
                stop_tensor_calc=stop,
                ins=[ifmap_ap, weights_ap],
                outs=[out_ap],
                tile_position=tile_position,
                tile_size=(64, 128),
                ldweights=False,
            )
        )

    with tile.TileContext(nc) as tc:
        with (
            tc.tile_pool(name="persist", bufs=1) as persist,
            tc.tile_pool(name="inp", bufs=3) as inp,
            tc.tile_pool(name="epool", bufs=8) as epool,
            tc.tile_pool(name="norm", bufs=3) as normp,
            tc.tile_pool(name="outp", bufs=2) as outp,
            tc.tile_pool(name="ps_sc", bufs=3, space="PSUM") as ps_sc,
            tc.tile_pool(name="ps_ctx", bufs=2, space="PSUM") as ps_ctx,
            tc.tile_pool(name="dramp", bufs=2, space="DRAM") as dramp,
        ):
            # ---- interleaved input/weight DMAs ----
            qT = persist.tile([65, 512], bf16, tag="qT")
            kT = persist.tile([65, 512], bf16, tag="kT")
            vT = persist.tile([65, 512], bf16, tag="vT")
            xins = {}
            wstgs = {}
            for name, x_d, w_d, b_d, eng in (
                ("q", q, qw_w, qw_b, nc.sync),
                ("k", k, kw_w, kw_b, nc.scalar),
                ("v", v, vw_w, vw_b, nc.gpsimd),
            ):
                xin = inp.tile([128, 4, 64], f32, tag="xin", name=f"xin_{name}")
                xr = x_d.rearrange("(t p) d -> p t d", p=128)
                for t in range(4):
                    eng.dma_start(out=xin[:, t, :], in_=xr[:, t, :])
                xins[name] = xin
                stg = inp.tile([65, 512], f32, tag="wstg", name=f"wstg_{name}")
                eng.dma_start(out=stg[0:64, :], in_=w_d)
                eng.dma_start(out=stg[64:65, :], in_=b_d[None, :])
                wstgs[name] = stg

            # prefetch output-projection weights (needed only at the tail,
            # but the DMA queues are idle here)
            ow_stg = persist.tile([64, 8, 64], f32, tag="ow_stg")
            nc.scalar.dma_start(
                out=ow_stg, in_=ow_w.rearrange("(c d) j -> d c j", d=64)
            )
            owb_stg = persist.tile([1, 64], f32, tag="owb_stg")
            nc.scalar.dma_start(out=owb_stg, in_=ow_b[None, :])

            # ---- constants (gpsimd/ACT, overlap the DMAs) ----
            ident = persist.tile([128, 128], f32, tag="ident")
            make_identity(nc, ident)
            ones_a = persist.tile([65, 64], f32, tag="ones_a")
            nc.gpsimd.memset(ones_a, 1.0)
            ones_b = persist.tile([1, 128], bf16, tag="ones_b")
            nc.gpsimd.memset(ones_b, 1.0)
            ones_row = persist.tile([1, 512], f32, tag="ones_row")
            nc.gpsimd.memset(ones_row, 1.0)
            # C3 latch for the custom exp op (exponent anchor per partition)
            anchor = persist.tile([128, 1], f32, tag="anchor")
            nc.gpsimd.memset(anchor, float(ANCHOR))
            # dummy exp to pull the ACT table load into the setup phase
            warm = persist.tile([1, 16], f32, tag="warm")
            nc.scalar.activation(warm, ones_row[:, 0:16], AF.Exp, scale=1.0)

            qw_aug = persist.tile([65, 512], bf16, tag="qw_aug")
            kw_aug = persist.tile([65, 512], bf16, tag="kw_aug")
            vw_aug = persist.tile([65, 512], bf16, tag="vw_aug")
            # weight casts only need the wstg DMAs -> run before the
            # transpose chains so they're off the projection critical path
            nc.vector.tensor_copy(out=qw_aug, in_=wstgs["q"])
            nc.vector.tensor_copy(out=kw_aug, in_=wstgs["k"])
            nc.vector.tensor_copy(out=vw_aug, in_=wstgs["v"])

            def transposes(name, xT):
                nc.vector.tensor_copy(out=xT[64:65, :], in_=ones_row)
                for t in range(4):
                    tp = ps_sc.tile([64, 128], f32, tag="sc", name=f"tp_{name}{t}")
                    nc.tensor.transpose(tp, xins[name][:, t, :], ident)
                    nc.vector.tensor_copy(
                        out=xT[0:64, 128 * t : 128 * t + 128], in_=tp
                    )

            # ---- Q chain: transposes, projections, chunk-0 dup ----
            # Qdup/KpT casts scale by SCP each so scores come out of the PE
            # pre-scaled by SCALE_C (code units for the DVE exp op).
            transposes("q", qT)

            Qdup = persist.tile([128, 4096], bf16, tag="Qdup")

            def qproj(m, eng):
                pool, tg = (ps_sc, "sc") if m % 2 == 0 else (ps_ctx, "ctx")
                ps = pool.tile([128, 512], f32, tag=tg, name=f"qp{m}")
                nc.tensor.matmul(
                    ps,
                    lhsT=qw_aug[:, 128 * m : 128 * m + 128],
                    rhs=qT[:],
                    start=True,
                    stop=True,
                )
                ce, co = 2 * m, 2 * m + 1
                dst_e = Qdup[0:64, 512 * ce : 512 * ce + 512]
                dst_o = Qdup[64:128, 512 * co : 512 * co + 512]
                if eng == "act":
                    nc.scalar.activation(dst_e, ps[0:64, :], AF.Copy, scale=SCP)
                    nc.vector.tensor_scalar_mul(dst_o, ps[64:128, :], SCP)
                else:
                    nc.vector.tensor_scalar_mul(dst_e, ps[0:64, :], SCP)
                    nc.scalar.activation(dst_o, ps[64:128, :], AF.Copy, scale=SCP)

            qproj(0, "dve")
            nc.sync.dma_start(out=Qdup[64:128, 0:512], in_=Qdup[0:64, 0:512])
            nc.sync.dma_start(out=Qdup[0:64, 512:1024], in_=Qdup[64:128, 512:1024])
            for m in range(1, 4):
                qproj(m, "act" if m % 2 == 0 else "dve")
                ce, co = 2 * m, 2 * m + 1
                nc.sync.dma_start(
                    out=Qdup[64:128, 512 * ce : 512 * ce + 512],
                    in_=Qdup[0:64, 512 * ce : 512 * ce + 512],
                )
                nc.sync.dma_start(
                    out=Qdup[0:64, 512 * co : 512 * co + 512],
                    in_=Qdup[64:128, 512 * co : 512 * co + 512],
                )

            # ---- K chain ----
            transposes("k", kT)
            KpT = []
            for g in range(4):
                pool, tg = (ps_sc, "sc") if g % 2 == 0 else (ps_ctx, "ctx")
                ps = pool.tile([128, 512], f32, tag=tg, name=f"kp{g}")
                nc.tensor.matmul(
                    ps,
                    lhsT=kw_aug[:, 128 * g : 128 * g + 128],
                    rhs=kT[:],
                    start=True,
                    stop=True,
                )
                sb = persist.tile([128, 512], bf16, tag=f"KpT{g}")
                if g % 2 == 0:
                    nc.scalar.activation(sb, ps, AF.Copy, scale=SCP)
                else:
                    nc.vector.tensor_scalar_mul(sb, ps, SCP)
                KpT.append(sb)

            # ---- V chain ----
            transposes("v", vT)
            Va = []
            for u in range(4):
                pool, tg = (ps_sc, "sc") if u % 2 == 0 else (ps_ctx, "ctx")
                ps = pool.tile([128, 512], f32, tag=tg, name=f"vp{u}")
                nc.tensor.matmul(
                    ps,
                    lhsT=vT[:, 128 * u : 128 * u + 128],
                    rhs=vw_aug[:],
                    start=True,
                    stop=True,
                )
                va = persist.tile([128, 520], bf16, tag=f"Va{u}")
                nc.gpsimd.memset(va, 1.0)
                vdst = va[:].rearrange("p (c jj) -> p c jj", c=8)[:, :, 0:64]
                vsrc = ps[:].rearrange("p (c j) -> p c j", c=8)
                if u % 2 == 0:
                    nc.scalar.copy(out=vdst, in_=vsrc)
                else:
                    nc.vector.tensor_copy(out=vdst, in_=vsrc)
                Va.append(va)

            # ---- main attention loop ----
            # units issued as packed pairs (kt=8g+u rows 0-63, kt=8g+4+u rows
            # 64-127); exp groups of gsize=2 units = [128, 1024] PSUM (2 banks,
            # 3 bufs so PE / ACT / DVE each own one in flight).
            unit_order = []
            for g in range(4):
                for u in range(4):
                    unit_order.append(8 * g + u)
                    unit_order.append(8 * g + 4 + u)

            ctxN = persist.tile([64, 4096], bf16, tag="ctxN")
            ctx_tiles = {}
            av_issued = {r1c: 0 for r1c in range(8)}
            pending = []  # (r1c, e_tile, units[(slot, kt)])
            AV_DELAY = 3
            gsize = 2

            dve_acc = [0]

            def pick_engine():
                dve_acc[0] += DVE_PATTERN_NUM
                if dve_acc[0] >= DVE_PATTERN_DEN:
                    dve_acc[0] -= DVE_PATTERN_DEN
                    return "dve"
                return "act"

            def emit_avs(rec_):
                r1c, e_bf, units = rec_
                ctx_ps = ctx_tiles[r1c]
                for slot, kt in units:
                    c, u = kt // 4, kt % 4
                    i = av_issued[r1c]
                    nc.tensor.matmul(
                        ctx_ps,
                        lhsT=Va[u][:, 65 * c : 65 * c + 65],
                        rhs=e_bf[:, 512 * slot : 512 * slot + 512],
                        start=(i == 0),
                        stop=(i == 31),
                    )
                    av_issued[r1c] = i + 1

            def normalize(r1c):
                # custom-DVE ops (reciprocal_approx_fast) are broken at
                # base_partition != 0, so broadcast the RAW denominator to a
                # base-0 [64, 512] first, then take the fast reciprocal there.
                ctx_ps = ctx_tiles.pop(r1c)
                den_sb = normp.tile([65, 512], f32, tag="densb")
                nc.vector.tensor_copy(out=den_sb[64:65, :], in_=ctx_ps[64:65, :])
                if r1c == 7:
                    # tail chunk: PE idle -> broadcast via K=1 ones matmul
                    den_bc_ps = ps_sc.tile([64, 512], f32, tag="sc", name="repl")
                    nc.tensor.matmul(
                        den_bc_ps,
                        lhsT=ones_a[64:65, :],
                        rhs=den_sb[64:65, :],
                        start=True,
                        stop=True,
                        tile_position=(64, 0),
                    )
                    den_src = den_bc_ps
                else:
                    rec_d = dramp.tile([1, 512], f32, tag="rec_d")
                    nc.sync.dma_start(out=rec_d, in_=den_sb[64:65, :])
                    den_bc = normp.tile([64, 512], f32, tag="denbc")
                    rd = rec_d[0, :]
                    nc.sync.dma_start(
                        out=den_bc,
                        in_=bass.AP(
                            tensor=rd.tensor,
                            offset=rd.offset,
                            ap=[[0, 64]] + list(rd.ap),
                        ),
                    )
                    den_src = den_bc
                rec_bc = normp.tile([64, 512], f32, tag="recbc")
                nc.vector.reciprocal_approx_fast(rec_bc, den_src)
                nc.vector.tensor_mul(
                    out=ctxN[:, 512 * r1c : 512 * r1c + 512],
                    in0=ctx_ps[0:64, :],
                    in1=rec_bc,
                )

            for r1c in range(8):
                ctx_tiles[r1c] = ps_ctx.tile(
                    [65, 512], f32, tag="ctx", name=f"ctx{r1c}"
                )
                group_tile = None
                group_units = []

                def flush(r1c=r1c):
                    nonlocal group_tile, group_units
                    if not group_units:
                        return
                    n = len(group_units)
                    e = epool.tile([128, 1024], bf16, tag="e")
                    if pick_engine() == "act":
                        nc.scalar.activation(
                            e[:, : 512 * n],
                            group_tile[:, : 512 * n],
                            AF.Exp,
                            scale=ACT_SCALE,
                        )
                    else:
                        nc.vector._custom_dve(
                            exp_op,
                            out=e[:, : 512 * n].bitcast(i16),
                            in0=group_tile[:, : 512 * n],
                            in1=anchor,
                            s0=QC,
                            s1=float(C1BIG),
                            imm2=LC,
                        )
                    pending.append((r1c, e, group_units))
                    group_tile = None
                    group_units = []
                    # drain four groups every fourth flush: fewer score<->AV
                    # transitions on the PE (each costs a ~100ns LDWEIGHTS
                    # bubble from row-group conflicts)
                    if len(pending) > AV_DELAY + 1:
                        for _ in range(2):
                            rec_ = pending.pop(0)
                            emit_avs(rec_)
                            if av_issued[rec_[0]] == 32:
                                normalize(rec_[0])

                for pi in range(16):
                    kt_a = unit_order[2 * pi]
                    kt_b = unit_order[2 * pi + 1]
                    for kt, half in ((kt_a, 0), (kt_b, 1)):
                        if group_tile is None:
                            group_tile = ps_sc.tile([128, 1024], f32, tag="sc")
                        slot = len(group_units)
                        c, u = kt // 4, kt % 4
                        g = c // 2
                        rowpos = 64 * (c % 2)
                        nc.tensor.matmul(
                            group_tile[:, 512 * slot : 512 * slot + 512],
                            lhsT=KpT[g][
                                rowpos : rowpos + 64, 128 * u : 128 * u + 128
                            ],
                            rhs=Qdup[rowpos : rowpos + 64, 512 * r1c : 512 * r1c + 512],
                            start=True,
                            stop=True,
                            tile_position=(rowpos, 0),
                        )
                        group_units.append((slot, kt))
                        if len(group_units) == gsize:
                            flush()
                flush()
            while pending:
                rec_ = pending.pop(0)
                emit_avs(rec_)
                if av_issued[rec_[0]] == 32:
                    normalize(rec_[0])

            # ---- output projection (bf16) ----
            ow_sb = persist.tile([64, 512], bf16, tag="ow_sb")
            nc.vector.tensor_copy(
                out=ow_sb, in_=ow_stg.rearrange("d c j -> d (c j)")
            )
            owb_sb = persist.tile([1, 64], bf16, tag="owb_sb")
            nc.vector.tensor_copy(out=owb_sb, in_=owb_stg)
            ob = outp.tile([128, 4, 64], f32, tag="ob")
            # t0..t2 accumulate chunks 0..6 in separate PSUM tiles while the
            # chunk-7 normalize chain runs on the DVE; only the final c=7
            # matmuls (and t3) wait on it.
            ops = {}
            for t in range(3):
                ops[t] = ps_sc.tile([128, 64], f32, tag="sc", name=f"op{t}")
                for c in range(7):
                    nc.tensor.matmul(
                        ops[t],
                        lhsT=ctxN[:, 512 * c + 128 * t : 512 * c + 128 * t + 128],
                        rhs=ow_sb[:, 64 * c : 64 * c + 64],
                        start=(c == 0),
                        stop=False,
                    )
            for t in range(3):
                nc.tensor.matmul(
                    ops[t],
                    lhsT=ctxN[:, 512 * 7 + 128 * t : 512 * 7 + 128 * t + 128],
                    rhs=ow_sb[:, 64 * 7 : 64 * 7 + 64],
                    start=False,
                    stop=False,
                )
                nc.tensor.matmul(
                    ops[t], lhsT=ones_b, rhs=owb_sb, start=False, stop=True
                )
                nc.vector.tensor_copy(out=ob[:, t, :], in_=ops[t])
            op3 = ps_sc.tile([128, 64], f32, tag="sc", name="op3")
            for c in range(8):
                nc.tensor.matmul(
                    op3,
                    lhsT=ctxN[:, 512 * c + 128 * 3 : 512 * c + 128 * 3 + 128],
                    rhs=ow_sb[:, 64 * c : 64 * c + 64],
                    start=(c == 0),
                    stop=False,
                )
            nc.tensor.matmul(
                op3, lhsT=ones_b, rhs=owb_sb, start=False, stop=True
            )
            nc.vector.tensor_copy(out=ob[:, 3, :], in_=op3)
            nc.sync.dma_start(
                out=out.rearrange("(t p) d -> p t d", p=128), in_=ob
            )

    nc.compile()
    return nc


def _get_built():
    global _BUILT
    if _BUILT is None:
        _BUILT = _build()
    return _BUILT


def _make_in_maps(inputs):
    f32 = np.float32
    full = {k: np.ascontiguousarray(np.asarray(v, dtype=f32)) for k, v in inputs.items()}
    in_maps = []
    for i in range(N_CORES):
        sl = slice(B * i, B * (i + 1))
        in_maps.append(
            {
                "q": full["q"][sl],
                "k": full["k"][sl],
                "v": full["v"][sl],
                "qw_w": full["qw_w"],
                "qw_b": full["qw_b"],
                "kw_w": full["kw_w"],
                "kw_b": full["kw_b"],
                "vw_w": full["vw_w"],
                "vw_b": full["vw_b"],
                "ow_w": full["ow_w"],
                "ow_b": full["ow_b"],
            }
        )
    return in_maps


def kernel(**inputs):
    # The jax persistent compilation cache has been observed to serve stale
    # executables across kernel revisions with identical I/O signatures
    # (the bass bir rides in frontend_attributes, which the cache fingerprint
    # can miss). Disable it -- the neuron NEFF cache (keyed on the bir bytes)
    # still makes recompiles cheap.
    import jax

    try:
        jax.config.update("jax_enable_compilation_cache", False)
    except Exception:
        pass
    from concourse.bass_utils import run_bass_kernel_spmd

    nc = _get_built()
    res = run_bass_kernel_spmd(nc, _make_in_maps(inputs), list(range(N_CORES)))
    return np.concatenate([res.results[i]["out"] for i in range(N_CORES)], axis=0)


# revision 27
# speedup vs baseline: 1.0071x; 1.0027x over previous
"""Trainium2 Bass kernel for nn_MultiHeadAttention_53266184405720.

Key structural fact: the reference does a raw ``.reshape(h, -1, d)`` on the
[4096, 512] projection output, so "head" h consumes exactly projection rows
[512h, 512h+512) — i.e. sequence rows [512h, 512h+512).  The whole module is
block-diagonal over 512-row sequence blocks: core h computes output rows
[512h, 512h+512) from input rows [512h, 512h+512) plus the (replicated)
weights.  No cross-core communication is needed.

Within a block, with the permutation r~ = c*512 + s (c = column-block of the
projection, s = row), head-reshaped Q/K/V become column-block stacks of the
projection, softmax is permutation-invariant over keys, and the context
unpermutes back into the output projection's contraction.  The transposed
projection layout [64, 512] per column-block c therefore yields every
attention operand as a zero-cost sub-AP.

Perf choices (HW-measured):
 - fp32 matmul = 4 cyc/row; bf16 = 1 cyc/row with fast weight loads -> bf16
   for scores / attention*V / output projection, f32r for input projections.
 - K=64 score matmuls pack 2-per-PE via tile_position rows (0,0)/(64,0).
 - THE EXP SPLIT: exp is the second wall (ACT = 1 elem/lane/cyc @1.2GHz =
   ~109us/core floor for 16.8M scores).  A custom DVE op (EXP2_BITS_ANT)
   computes exp as int16 bf16-bit-pattern: scores arrive pre-scaled by
   16*log2(e) (folded into the Q/K casts), the op extracts the binade
   fraction with the fp32 big-constant RNE trick (+1.5*2^30), applies a
   minimax quadratic mantissa correction (~0.9% rms), and the i16 result is
   bit-cast to bf16.  Score groups alternate ACT (exact exp) / DVE (custom
   op) so both engines chew the exp wall concurrently.
 - softmax denominator rides as a ones-column in the V operand (row 64 of
   the ctx accumulator); the RAW denominator is broadcast to a base-0
   [64, 512] (DRAM-bounce DMA; K=1 ones-matmul for the tail chunk), then
   1/denom via reciprocal_approx_fast (custom-DVE ops only work at
   base_partition 0; ~5x faster than the exact iterative reciprocal).
 - AV emission batched two exp-groups at a time: each score<->AV switch on
   the PE pays a ~100-200ns LDWEIGHTS bubble (tile-positioned weight loads
   cannot use the background weight buffer), so fewer switches = less idle.
 - fp8/DoubleRow was evaluated and rejected: e4m3 quantization of the
   attention weights alone gives 7e-2 end-to-end (scores span +-11 sigma,
   so no global shift fits e4m3's range next to its 3-bit mantissa).
 - out-projection: three of the four row-block accumulators process chunks
   0..6 while the chunk-7 normalize chain still runs on the DVE.
"""

import numpy as np

SEQ = 4096
D = 64
HEADS = 8
B = SEQ // HEADS  # 512 rows per core
N_CORES = 8

_BUILT = None
_EXP_OP = None

# exp-op constants: scores arrive pre-scaled so that score_c = 16*log2e * s_raw
# (exp argument is s_raw/8 -> w_code = 128 * log2(exp-arg... ) = score_c).
SCALE_C = 16.0 * np.log2(np.e)          # 23.083120654223414
SCP = float(np.sqrt(SCALE_C))           # folded into both Q and K casts
C1BIG = 1.5 * 2**30
# minimax quadratic fit of the round-based (kinked) mantissa correction
QC = -0.00247243   # quadratic coef (s0 / C0)
LC = 0.013384      # linear coef (imm2 / C2)
CC = -2.8858       # constant coef -> folded into the exponent anchor (C3)
ANCHOR = 16256.0 + CC

DVE_PATTERN_NUM = 6    # DVE gets 6 of every 16 exp groups
DVE_PATTERN_DEN = 16


def _get_exp_op():
    global _EXP_OP
    if _EXP_OP is not None:
        return _EXP_OP
    import dataclasses
    from concourse.dve_spec import Spec, Src0, C0, C1, C2, C3, lower
    from concourse.dve_ops import (
        DveOp, OPS, CUSTOM_DVE_SPECS, _SUB_OPCODE_FOR_NAME,
        _CUSTOM_DVE_ROW_BASE, DveOpSpec, _spill_c3_to_src1,
    )
    name = "EXP2_BITS_ANT"
    if name in _SUB_OPCODE_FOR_NAME:
        _EXP_OP = next(o for o in OPS if o.name == name)
        return _EXP_OP
    _w = Src0 + C1
    _v = _w - C1              # RNE to multiples of 128 (fp32 big-const trick)
    _g = Src0 - _v            # signed binade frac in code units, [-64, 64)
    _p = _g * (_g * C0 + C2)  # quadratic mantissa correction
    _t = (Src0 + _p) + C3     # C3 (latched from in1) = exponent anchor

    def _ref(in0, in1, s0, s1, imm2):
        w = (in0 + np.float32(s1)).astype(np.float32)
        v = (w - np.float32(s1)).astype(np.float32)
        g = in0 - v
        c3 = in1[:, :1] if in1 is not None else 0.0
        return (in0 + g * (g * np.float32(s0) + np.float32(imm2))) + c3

    op = DveOp(name, Spec(body=_spill_c3_to_src1(_t), reference=_ref),
               subdim=False, uops_sha={})
    row = _CUSTOM_DVE_ROW_BASE + len(OPS)
    shas = {}
    for ver in ("v3", "v4"):
        spec_l = DveOpSpec(name=name, opcode=row,
                           uops=lower(op.spec, ver=ver), rd1_en=True)
        shas[ver] = spec_l.sha(ver)
    op = dataclasses.replace(op, uops_sha=shas)
    OPS.append(op)
    CUSTOM_DVE_SPECS[name] = op.spec
    _SUB_OPCODE_FOR_NAME[name] = row
    _EXP_OP = op
    return op


def _build():
    import concourse.bass as bass
    import concourse.tile as tile
    from concourse import bacc, mybir
    from concourse.masks import make_identity

    exp_op = _get_exp_op()

    f32 = mybir.dt.float32
    f32r = mybir.dt.float32r
    bf16 = mybir.dt.bfloat16
    i16 = mybir.dt.int16
    AF = mybir.ActivationFunctionType

    nc = bacc.Bacc(
        "TRN2",
        target_bir_lowering=False,
        debug=False,
        enable_asserts=True,
        num_devices=N_CORES,
    )

    q = nc.dram_tensor("q", [B, D], f32, kind="ExternalInput").ap()
    k = nc.dram_tensor("k", [B, D], f32, kind="ExternalInput").ap()
    v = nc.dram_tensor("v", [B, D], f32, kind="ExternalInput").ap()
    qw_w = nc.dram_tensor("qw_w", [D, 512], f32, kind="ExternalInput").ap()
    qw_b = nc.dram_tensor("qw_b", [512], f32, kind="ExternalInput").ap()
    kw_w = nc.dram_tensor("kw_w", [D, 512], f32, kind="ExternalInput").ap()
    kw_b = nc.dram_tensor("kw_b", [512], f32, kind="ExternalInput").ap()
    vw_w = nc.dram_tensor("vw_w", [D, 512], f32, kind="ExternalInput").ap()
    vw_b = nc.dram_tensor("vw_b", [512], f32, kind="ExternalInput").ap()
    ow_w = nc.dram_tensor("ow_w", [512, D], f32, kind="ExternalInput").ap()
    ow_b = nc.dram_tensor("ow_b", [D], f32, kind="ExternalInput").ap()
    out = nc.dram_tensor("out", [B, D], f32, kind="ExternalOutput").ap()

    ACT_SCALE = float(0.125 / SCALE_C)  # exp(scale * score_c) = exp(s/8)

    def matmul_noldw(out, lhsT, rhs, start, stop, tile_position):
        """InstMatmult with ldweights=False: weights must already be resident
        (loaded by a preceding nc.tensor.ldweights covering this tile)."""
        te = nc.tensor
        keep_dims = frozenset({0})
        ifmap_ap = te.lower_ap(rhs.opt(keep_dims), opt=False)
        weights_ap = te.lower_ap(lhsT.opt(keep_dims), opt=False, for_matmul_weights=True)
        out_ap = te.lower_ap(out)
        return te.add_instruction(
            mybir.InstMatmult(
                name=nc.get_next_instruction_name(),
                replication_resolution=0,
                replication_shift_amnt=0,
                replication_num_rows=0,
                start_tensor_calc=start,
                stop_tensor_calc=stop,
                ins=[ifmap_ap, weights_ap],
                outs=[out_ap],
                tile_position=tile_position,
                tile_size=(64, 128),
                ldweights=False,
            )
        )

    with tile.TileContext(nc) as tc:
        with (
            tc.tile_pool(name="persist", bufs=1) as persist,
            tc.tile_pool(name="inp", bufs=3) as inp,
            tc.tile_pool(name="epool", bufs=8) as epool,
            tc.tile_pool(name="norm", bufs=3) as normp,
            tc.tile_pool(name="outp", bufs=2) as outp,
            tc.tile_pool(name="ps_sc", bufs=3, space="PSUM") as ps_sc,
            tc.tile_pool(name="ps_ctx", bufs=2, space="PSUM") as ps_ctx,
            tc.tile_pool(name="dramp", bufs=2, space="DRAM") as dramp,
        ):
            # ---- interleaved input/weight DMAs ----
            qT = persist.tile([65, 512], bf16, tag="qT")
            kT = persist.tile([65, 512], bf16, tag="kT")
            vT = persist.tile([65, 512], bf16, tag="vT")
            xins = {}
            wstgs = {}
            for name, x_d, w_d, b_d, eng in (
                ("q", q, qw_w, qw_b, nc.sync),
                ("k", k, kw_w, kw_b, nc.scalar),
                ("v", v, vw_w, vw_b, nc.gpsimd),
            ):
                xin = inp.tile([128, 4, 64], f32, tag="xin", name=f"xin_{name}")
                xr = x_d.rearrange("(t p) d -> p t d", p=128)
                for t in range(4):
                    eng.dma_start(out=xin[:, t, :], in_=xr[:, t, :])
                xins[name] = xin
                stg = inp.tile([65, 512], f32, tag="wstg", name=f"wstg_{name}")
                eng.dma_start(out=stg[0:64, :], in_=w_d)
                eng.dma_start(out=stg[64:65, :], in_=b_d[None, :])
                wstgs[name] = stg

            # prefetch output-projection weights (needed only at the tail,
            # but the DMA queues are idle here)
            ow_stg = persist.tile([64, 8, 64], f32, tag="ow_stg")
            nc.scalar.dma_start(
                out=ow_stg, in_=ow_w.rearrange("(c d) j -> d c j", d=64)
            )
            owb_stg = persist.tile([1, 64], f32, tag="owb_stg")
            nc.scalar.dma_start(out=owb_stg, in_=ow_b[None, :])

            # ---- constants (gpsimd/ACT, overlap the DMAs) ----
            ident = persist.tile([128, 128], f32, tag="ident")
            make_identity(nc, ident)
            ones_a = persist.tile([65, 64], f32, tag="ones_a")
            nc.gpsimd.memset(ones_a, 1.0)
            ones_b = persist.tile([1, 128], bf16, tag="ones_b")
            nc.gpsimd.memset(ones_b, 1.0)
            ones_row = persist.tile([1, 512], f32, tag="ones_row")
            nc.gpsimd.memset(ones_row, 1.0)
            # C3 latch for the custom exp op (exponent anchor per partition)
            anchor = persist.tile([128, 1], f32, tag="anchor")
            nc.gpsimd.memset(anchor, float(ANCHOR))
            # dummy exp to pull the ACT table load into the setup phase
            warm = persist.tile([1, 16], f32, tag="warm")
            nc.scalar.activation(warm, ones_row[:, 0:16], AF.Exp, scale=1.0)

            qw_aug = persist.tile([65, 512], bf16, tag="qw_aug")
            kw_aug = persist.tile([65, 512], bf16, tag="kw_aug")
            vw_aug = persist.tile([65, 512], bf16, tag="vw_aug")
            # weight casts only need the wstg DMAs -> run before the
            # transpose chains so they're off the projection critical path
            nc.vector.tensor_copy(out=qw_aug, in_=wstgs["q"])
            nc.vector.tensor_copy(out=kw_aug, in_=wstgs["k"])
            nc.vector.tensor_copy(out=vw_aug, in_=wstgs["v"])

            def transposes(name, xT):
                nc.vector.tensor_copy(out=xT[64:65, :], in_=ones_row)
                for t in range(4):
                    tp = ps_sc.tile([64, 128], f32, tag="sc", name=f"tp_{name}{t}")
                    nc.tensor.transpose(tp, xins[name][:, t, :], ident)
                    nc.vector.tensor_copy(
                        out=xT[0:64, 128 * t : 128 * t + 128], in_=tp
                    )

            # ---- Q chain: transposes, projections, chunk-0 dup ----
            # Qdup/KpT casts scale by SCP each so scores come out of the PE
            # pre-scaled by SCALE_C (code units for the DVE exp op).
            transposes("q", qT)

            Qdup = persist.tile([128, 4096], bf16, tag="Qdup")

            def qproj(m, eng):
                pool, tg = (ps_sc, "sc") if m % 2 == 0 else (ps_ctx, "ctx")
                ps = pool.tile([128, 512], f32, tag=tg, name=f"qp{m}")
                nc.tensor.matmul(
                    ps,
                    lhsT=qw_aug[:, 128 * m : 128 * m + 128],
                    rhs=qT[:],
                    start=True,
                    stop=True,
                )
                ce, co = 2 * m, 2 * m + 1
                dst_e = Qdup[0:64, 512 * ce : 512 * ce + 512]
                dst_o = Qdup[64:128, 512 * co : 512 * co + 512]
                if eng == "act":
                    nc.scalar.activation(dst_e, ps[0:64, :], AF.Copy, scale=SCP)
                    nc.vector.tensor_scalar_mul(dst_o, ps[64:128, :], SCP)
                else:
                    nc.vector.tensor_scalar_mul(dst_e, ps[0:64, :], SCP)
                    nc.scalar.activation(dst_o, ps[64:128, :], AF.Copy, scale=SCP)

            qproj(0, "dve")
            nc.sync.dma_start(out=Qdup[64:128, 0:512], in_=Qdup[0:64, 0:512])
            nc.sync.dma_start(out=Qdup[0:64, 512:1024], in_=Qdup[64:128, 512:1024])
            for m in range(1, 4):
                qproj(m, "act" if m % 2 == 0 else "dve")
                ce, co = 2 * m, 2 * m + 1
                nc.sync.dma_start(
                    out=Qdup[64:128, 512 * ce : 512 * ce + 512],
                    in_=Qdup[0:64, 512 * ce : 512 * ce + 512],
                )
                nc.sync.dma_start(
                    out=Qdup[0:64, 512 * co : 512 * co + 512],
                    in_=Qdup[64:128, 512 * co : 512 * co + 512],
                )

            # ---- K chain ----
            transposes("k", kT)
            KpT = []
            for g in range(4):
                pool, tg = (ps_sc, "sc") if g % 2 == 0 else (ps_ctx, "ctx")
                ps = pool.tile([128, 512], f32, tag=tg, name=f"kp{g}")
                nc.tensor.matmul(
                    ps,
                    lhsT=kw_aug[:, 128 * g : 128 * g + 128],
                    rhs=kT[:],
                    start=True,
                    stop=True,
                )
                sb = persist.tile([128, 512], bf16, tag=f"KpT{g}")
                if g % 2 == 0:
                    nc.scalar.activation(sb, ps, AF.Copy, scale=SCP)
                else:
                    nc.vector.tensor_scalar_mul(sb, ps, SCP)
                KpT.append(sb)

            # ---- V chain ----
            transposes("v", vT)
            Va = []
            for u in range(4):
                pool, tg = (ps_sc, "sc") if u % 2 == 0 else (ps_ctx, "ctx")
                ps = pool.tile([128, 512], f32, tag=tg, name=f"vp{u}")
                nc.tensor.matmul(
                    ps,
                    lhsT=vT[:, 128 * u : 128 * u + 128],
                    rhs=vw_aug[:],
                    start=True,
                    stop=True,
                )
                va = persist.tile([128, 520], bf16, tag=f"Va{u}")
                nc.gpsimd.memset(va, 1.0)
                vdst = va[:].rearrange("p (c jj) -> p c jj", c=8)[:, :, 0:64]
                vsrc = ps[:].rearrange("p (c j) -> p c j", c=8)
                if u % 2 == 0:
                    nc.scalar.copy(out=vdst, in_=vsrc)
                else:
                    nc.vector.tensor_copy(out=vdst, in_=vsrc)
                Va.append(va)

            # ---- main attention loop ----
            # units issued as packed pairs (kt=8g+u rows 0-63, kt=8g+4+u rows
            # 64-127); exp groups of gsize=2 units = [128, 1024] PSUM (2 banks,
            # 3 bufs so PE / ACT / DVE each own one in flight).
            unit_order = []
            for g in range(4):
                for u in range(4):
                    unit_order.append(8 * g + u)
                    unit_order.append(8 * g + 4 + u)

            ctxN = persist.tile([64, 4096], bf16, tag="ctxN")
            ctx_tiles = {}
            av_issued = {r1c: 0 for r1c in range(8)}
            pending = []  # (r1c, e_tile, units[(slot, kt)])
            AV_DELAY = 3
            gsize = 2

            dve_acc = [0]

            def pick_engine():
                dve_acc[0] += DVE_PATTERN_NUM
                if dve_acc[0] >= DVE_PATTERN_DEN:
                    dve_acc[0] -= DVE_PATTERN_DEN
                    return "dve"
                return "act"

            def emit_avs(rec_):
                r1c, e_bf, units = rec_
                ctx_ps = ctx_tiles[r1c]
                for slot, kt in units:
                    c, u = kt // 4, kt % 4
                    i = av_issued[r1c]
                    nc.tensor.matmul(
                        ctx_ps,
                        lhsT=Va[u][:, 65 * c : 65 * c + 65],
                        rhs=e_bf[:, 512 * slot : 512 * slot + 512],
                        start=(i == 0),
                        stop=(i == 31),
                    )
                    av_issued[r1c] = i + 1

            def normalize(r1c):
                # custom-DVE ops (reciprocal_approx_fast) are broken at
                # base_partition != 0, so broadcast the RAW denominator to a
                # base-0 [64, 512] first, then take the fast reciprocal there.
                ctx_ps = ctx_tiles.pop(r1c)
                den_sb = normp.tile([65, 512], f32, tag="densb")
                nc.vector.tensor_copy(out=den_sb[64:65, :], in_=ctx_ps[64:65, :])
                if r1c == 7:
                    # tail chunk: PE idle -> broadcast via K=1 ones matmul
                    den_bc_ps = ps_sc.tile([64, 512], f32, tag="sc", name="repl")
                    nc.tensor.matmul(
                        den_bc_ps,
                        lhsT=ones_a[64:65, :],
                        rhs=den_sb[64:65, :],
                        start=True,
                        stop=True,
                        tile_position=(64, 0),
                    )
                    den_src = den_bc_ps
                else:
                    rec_d = dramp.tile([1, 512], f32, tag="rec_d")
                    nc.sync.dma_start(out=rec_d, in_=den_sb[64:65, :])
                    den_bc = normp.tile([64, 512], f32, tag="denbc")
                    rd = rec_d[0, :]
                    nc.sync.dma_start(
                        out=den_bc,
                        in_=bass.AP(
                            tensor=rd.tensor,
                            offset=rd.offset,
                            ap=[[0, 64]] + list(rd.ap),
                        ),
                    )
                    den_src = den_bc
                rec_bc = normp.tile([64, 512], f32, tag="recbc")
                nc.vector.reciprocal_approx_fast(rec_bc, den_src)
                nc.vector.tensor_mul(
                    out=ctxN[:, 512 * r1c : 512 * r1c + 512],
                    in0=ctx_ps[0:64, :],
                    in1=rec_bc,
                )

            for r1c in range(8):
                ctx_tiles[r1c] = ps_ctx.tile(
                    [65, 512], f32, tag="ctx", name=f"ctx{r1c}"
                )
                group_tile = None
                group_units = []

                def flush(r1c=r1c):
                    nonlocal group_tile, group_units
                    if not group_units:
                        return
                    n = len(group_units)
                    e = epool.tile([128, 1024], bf16, tag="e")
                    if pick_engine() == "act":
                        nc.scalar.activation(
                            e[:, : 512 * n],
                            group_tile[:, : 512 * n],
                            AF.Exp,
                            scale=ACT_SCALE,
                        )
                    else:
                        nc.vector._custom_dve(
                            exp_op,
                            out=e[:, : 512 * n].bitcast(i16),
                            in0=group_tile[:, : 512 * n],
                            in1=anchor,
                            s0=QC,
                            s1=float(C1BIG),
                            imm2=LC,
                        )
                    pending.append((r1c, e, group_units))
                    group_tile = None
                    group_units = []
                    # drain four groups every fourth flush: fewer score<->AV
                    # transitions on the PE (each costs a ~100ns LDWEIGHTS
                    # bubble from row-group conflicts)
                    if len(pending) > AV_DELAY + 1:
                        for _ in range(2):
                            rec_ = pending.pop(0)
                            emit_avs(rec_)
                            if av_issued[rec_[0]] == 32:
                                normalize(rec_[0])

                for pi in range(16):
                    kt_a = unit_order[2 * pi]
                    kt_b = unit_order[2 * pi + 1]
                    for kt, half in ((kt_a, 0), (kt_b, 1)):
                        if group_tile is None:
                            group_tile = ps_sc.tile([128, 1024], f32, tag="sc")
                        slot = len(group_units)
                        c, u = kt // 4, kt % 4
                        g = c // 2
                        rowpos = 64 * (c % 2)
                        nc.tensor.matmul(
                            group_tile[:, 512 * slot : 512 * slot + 512],
                            lhsT=KpT[g][
                                rowpos : rowpos + 64, 128 * u : 128 * u + 128
                            ],
                            rhs=Qdup[rowpos : rowpos + 64, 512 * r1c : 512 * r1c + 512],
                            start=True,
                            stop=True,
                            tile_position=(rowpos, 0),
                        )
                        group_units.append((slot, kt))
                        if len(group_units) == gsize:
                            flush()
                flush()
            while pending:
                rec_ = pending.pop(0)
                emit_avs(rec_)
                if av_issued[rec_[0]] == 32:
                    normalize(rec_[0])

            # ---- output projection (bf16) ----
            ow_sb = persist.tile([64, 512], bf16, tag="ow_sb")
            nc.vector.tensor_copy(
                out=ow_sb, in_=ow_stg.rearrange("d c j -> d (c j)")
            )
            owb_sb = persist.tile([1, 64], bf16, tag="owb_sb")
            nc.vector.tensor_copy(out=owb_sb, in_=owb_stg)
            ob = outp.tile([128, 4, 64], f32, tag="ob")
            # t0..t2 accumulate chunks 0..6 in separate PSUM tiles while the
            # chunk-7 normalize chain runs on the DVE; only the final c=7
            # matmuls (and t3) wait on it.
            ops = {}
            for t in range(3):
                ops[t] = ps_sc.tile([128, 64], f32, tag="sc", name=f"op{t}")
                for c in range(7):
                    nc.tensor.matmul(
                        ops[t],
                        lhsT=ctxN[:, 512 * c + 128 * t : 512 * c + 128 * t + 128],
                        rhs=ow_sb[:, 64 * c : 64 * c + 64],
                        start=(c == 0),
                        stop=False,
                    )
            for t in range(3):
                nc.tensor.matmul(
                    ops[t],
                    lhsT=ctxN[:, 512 * 7 + 128 * t : 512 * 7 + 128 * t + 128],
                    rhs=ow_sb[:, 64 * 7 : 64 * 7 + 64],
                    start=False,
                    stop=False,
                )
                nc.tensor.matmul(
                    ops[t], lhsT=ones_b, rhs=owb_sb, start=False, stop=True
                )
                nc.vector.tensor_copy(out=ob[:, t, :], in_=ops[t])
            op3 = ps_sc.tile([128, 64], f32, tag="sc", name="op3")
            for c in range(8):
                nc.tensor.matmul(
                    op3,
                    lhsT=ctxN[:, 512 * c + 128 * 3 : 512 * c + 128 * 3 + 128],
                    rhs=ow_sb[:, 64 * c : 64 * c + 64],
                    start=(c == 0),
                    stop=False,
                )
            nc.tensor.matmul(
                op3, lhsT=ones_b, rhs=owb_sb, start=False, stop=True
            )
            nc.vector.tensor_copy(out=ob[:, 3, :], in_=op3)
            nc.sync.dma_start(
                out=out.rearrange("(t p) d -> p t d", p=128), in_=ob
            )

    nc.compile()
    return nc


def _get_built():
    global _BUILT
    if _BUILT is None:
        _BUILT = _build()
    return _BUILT


def _make_in_maps(inputs):
    f32 = np.float32
    full = {k: np.ascontiguousarray(np.asarray(v, dtype=f32)) for k, v in inputs.items()}
    in_maps = []
    for i in range(N_CORES):
        sl = slice(B * i, B * (i + 1))
        in_maps.append(
            {
                "q": full["q"][sl],
                "k": full["k"][sl],
                "v": full["v"][sl],
                "qw_w": full["qw_w"],
                "qw_b": full["qw_b"],
                "kw_w": full["kw_w"],
                "kw_b": full["kw_b"],
                "vw_w": full["vw_w"],
                "vw_b": full["vw_b"],
                "ow_w": full["ow_w"],
                "ow_b": full["ow_b"],
            }
        )
    return in_maps


def kernel(**inputs):
    # The jax persistent compilation cache has been observed to serve stale
    # executables across kernel revisions with identical I/O signatures
    # (the bass bir rides in frontend_attributes, which the cache fingerprint
    # can miss). Disable it -- the neuron NEFF cache (keyed on the bir bytes)
    # still makes recompiles cheap.
    import jax

    try:
        jax.config.update("jax_enable_compilation_cache", False)
    except Exception:
        pass
    from concourse.bass_utils import run_bass_kernel_spmd

    nc = _get_built()
    res = run_bass_kernel_spmd(nc, _make_in_maps(inputs), list(range(N_CORES)))
    return np.concatenate([res.results[i]["out"] for i in range(N_CORES)], axis=0)
